# revision 22
# baseline (speedup 1.0000x reference)
"""Trainium2 Bass kernel for nn_Attention_org_single_85074712199391.

Channel-attention module. Reference math (per batch b, head h):
    Qc = emb1[b].reshape(N, 4, dq)[:, h]          # [N, 128]
    Kc = emb_all[b].reshape(N, 4, dk)[:, h]       # [N, 240]
    Q = Qc @ Wq[h].T ; K = Kc @ Wk.T ; V = Kc @ Wv.T
    scores = Q.T @ K / sqrt(KV)                   # [128, 240]
    probs = softmax(instnorm(scores), axis=-1)
    context = probs @ V.T                         # [128, N]
    O1 = permute/concat(context) @ Wo.T           # [N, 512]

Algebraic rewrite used here (exact):
    S_h      = Qc.T @ Kc                          # big contraction over N
    scores_h = (Wq[h]/sqrt(KV)) @ S_h @ Wk.T
    probs_h  = softmax over dk of rstd*scores_h   # mean cancels in softmax
    P2_h     = probs_h @ Wv                       # [128, 240], unnormalized
    ctx_h    = (P2_h @ Kc.T) / den_h              # [128, N]
    O1       = sum_h ctx_h.T @ Wo[:, h::4].T      # accumulate over heads

Per core (core b owns batch b; weights replicated; no collectives):
    A: stream e1/ea (host-converted fp16, pre-tiled); accumulate S in two
       2-head-packed PSUM banks.  The transposed-ea tiles (eaT) that phase C
       needs come from a MIX of host-pre-transposed DMA loads (HOST_EAT
       chunks, spread across the SP/ACT/gpsimd DMA queues) and on-chip PE
       transposes (remaining chunks; PSUM evac alternates DVE/ACT).
    B: fp16 scores-path matmuls (S/Wq/Wk/U quantized to fp16; the score
       values themselves stay fp32 through PSUM/SBUF so the softmax input
       is accurate); instance-norm stats via ones-matmul with the x^2 row
       sums on DVE; softmax denominator deferred to the phase C ctx evac.
    C: context matmuls off eaT, output projection accumulating over heads,
       fp16 stores batched per 512-token block on the gpsimd (SWDGE) queue.

Rep pipelining: rep r+1's phase A instruction stream is emitted
interleaved with rep r's phases B+C, so each engine's in-order queue
always holds cross-rep work while serial chains (instance-norm stats,
softmax) block the current rep. eaT chunk tiles give write-after-read
dependencies at chunk granularity (phase A block ii <-> phase C chunk ii).
"""

import sys

import numpy as np

try:
    import concourse.bass as bass
except ImportError:  # harness environments without the repo on sys.path
    sys.path.insert(0, "/opt/trn_rl_repo")
    import concourse.bass as bass

import concourse.bacc as bacc

import concourse.mybir as mybir
import concourse.tile as tile
from concourse.bass_utils import run_bass_kernel_spmd

import os as _os0

F32 = mybir.dt.float32
if _os0.environ.get("DT16", "f16") == "bf16":
    import ml_dtypes
    F16 = mybir.dt.bfloat16
    NP16 = ml_dtypes.bfloat16
else:
    F16 = mybir.dt.float16
    NP16 = np.float16
AF = mybir.ActivationFunctionType
ALU = mybir.AluOpType

B, N, C, KV, H = 8, 4096, 512, 960, 4
DQ, DK = C // 4, KV // 4          # 128, 240
PT = 128                          # partition tile
NT = N // PT                      # 32 row tiles
NCH = N // 512                    # 8 column chunks / token blocks
DCH = 8                           # KV split into 8 chunks of 120 partitions
CHW = KV // DCH                   # 120
KCH = 2                           # dk split for 240-deep contractions
KHW = DK // KCH                   # 120
EPS = 1e-5
NORM_CNT = float(DQ * DK)         # instance-norm element count

import os as _os
# token blocks whose eaT chunk is host-pre-transposed and DMA-loaded
# (trades spare DMA bandwidth for PE transpose + PSUM-evac time)
HOST_EAT = int(_os.environ.get("HOST_EAT", "4"))
HOST_SET = {round((2 * k + 1) * NCH / (2 * HOST_EAT) - 0.5) for k in range(HOST_EAT)} if HOST_EAT else set()
# DMA queue per host-transposed chunk, cycled: s=SP, a=ACT, p=gpsimd/Pool
EATP_QS = _os.environ.get("EATP_QS", "spsp")
# queue for the batched per-block output stores: s/a/p
STORE_Q = _os.environ.get("STORE_Q", "p")
# compute rstd = 1/sqrt(var) on DVE (magic-seed Newton) instead of ACT Sqrt;
# keeps the ACT exp table resident forever (no per-rep table churn)
RSQRT_DVE = _os.environ.get("RSQRT_DVE", "0") == "1"
# double-buffer the host-loaded eaT chunks so rep r+1's eatp DMA never
# waits on rep r's phase-C reads (costs 8KB/partition per chunk)
EAT_DB = _os.environ.get("EAT_DB", "0") == "1"
# 1 = one DMA per 512-token block; 0 = one DMA per 128-row tile (4x more)
STORE_BATCH = _os.environ.get("STORE_BATCH", "1") == "1"


def build_nc(reps=1):
    nc = bacc.Bacc("TRN2", target_bir_lowering=False, debug=False)

    # pre-tiled fp16 inputs: [8 blocks][128 partitions][4 subtiles][ch]
    e1 = nc.dram_tensor("e1", [NCH, PT, 4, C], F16, kind="ExternalInput").ap()
    ea = nc.dram_tensor("ea", [NCH, PT, 4, KV], F16, kind="ExternalInput").ap()
    wqt = nc.dram_tensor("wqt", [DQ, H, DQ], F16, kind="ExternalInput").ap()
    wkt = nc.dram_tensor("wkt", [DK, DK], F16, kind="ExternalInput").ap()
    wvb = nc.dram_tensor("wvb", [DK, DK], F16, kind="ExternalInput").ap()
    wotb = nc.dram_tensor("wotb", [DQ, H, C], F16, kind="ExternalInput").ap()
    idb = nc.dram_tensor("idb", [PT, PT], F16, kind="ExternalInput").ap()
    eatp = nc.dram_tensor("eatp", [max(len(HOST_SET), 1), CHW, DCH, 512],
                          F16, kind="ExternalInput").ap()
    # output stored block-tiled: token n = nch*512 + t*128 + p -> [nch, p, t, :]
    o1 = nc.dram_tensor("o1", [NCH, PT, 4, C], F16, kind="ExternalOutput").ap()

    from contextlib import ExitStack

    with tile.TileContext(nc) as tc, ExitStack() as stk:
        ent = stk.enter_context
        pW = ent(tc.tile_pool(name="weights", bufs=1))
        pEAT = ent(tc.tile_pool(name="eaTbuf", bufs=1))
        pP = ent(tc.tile_pool(name="persist", bufs=2))
        pA = ent(tc.tile_pool(name="pA", bufs=int(_os.environ.get('PA_BUFS', '7'))))
        pBs = ent(tc.tile_pool(name="pBs", bufs=int(_os.environ.get('PBS_BUFS', '2'))))
        pC = ent(tc.tile_pool(name="pC", bufs=int(_os.environ.get('PC_BUFS', '3'))))
        psS = ent(tc.tile_pool(name="psS", bufs=1, space="PSUM"))
        psT = ent(tc.tile_pool(name="psT", bufs=int(_os.environ.get('PST_BUFS','2')), space="PSUM"))
        psB = ent(tc.tile_pool(name="psB", bufs=int(_os.environ.get('PSB_BUFS','2')), space="PSUM"))
        psC = ent(tc.tile_pool(name="psC", bufs=int(_os.environ.get('PSC_BUFS','2')), space="PSUM"))

        # --- persistent weights / constants (loaded once) -----------------
        wqt_sb = pW.tile([DQ, H, DQ], F16, tag="wqt_sb")
        nc.sync.dma_start(wqt_sb[:], wqt[:])
        wkt_sb = pW.tile([KHW, KCH, DK], F16, tag="wkt_sb")
        wvb_sb = pW.tile([KHW, KCH, DK], F16, tag="wvb_sb")
        for j in range(KCH):
            nc.sync.dma_start(wkt_sb[:, j, :], wkt[j * KHW:(j + 1) * KHW, :])
            nc.sync.dma_start(wvb_sb[:, j, :], wvb[j * KHW:(j + 1) * KHW, :])
        wotb_sb = pW.tile([DQ, H, C], F16, tag="wotb_sb")
        nc.sync.dma_start(wotb_sb[:], wotb[:])
        idb_sb = pW.tile([PT, PT], F16, tag="idb_sb")
        nc.sync.dma_start(idb_sb[:], idb[:])
        ones_sb = pW.tile([PT, PT], F32, tag="ones_sb")
        nc.vector.memset(ones_sb[:], 1.0)
        dumm_sb = pW.tile([PT, 1], F32, tag="dumm_sb")
        nc.vector.memset(dumm_sb[:], 1.0)
        expd_sb = pW.tile([PT, 1], F32, tag="expd_sb")
        # one-time exp-table preload; every ACT func used afterwards
        # (Copy/Square/Exp) lives in the exp_and_friends table
        nc.scalar.activation(expd_sb[:], dumm_sb[:], AF.Exp)
        magic_sb = pW.tile([DQ, H], mybir.dt.int32, tag="magic_sb")
        # 0x5F3759DF: rsqrt magic seed (see make_state/gen_phase_bc)
        nc.vector.memset(magic_sb[:], 0x5F3759DF)
        env_magic = magic_sb

        # eaT is split per 512-token chunk: phase A block ii fills chunk ii
        # and phase C chunk ii is its only reader, so rep r+1's writes only
        # wait for rep r's same-chunk reads.  Host-loaded chunks may be
        # double-buffered (EAT_DB) to decouple that dependency entirely.
        def eaT_tile(nch):
            db = EAT_DB and nch in HOST_SET
            return pEAT.tile([CHW, DCH, 512], F16, tag=f"eaT{nch}",
                             name=f"eaT{nch}", bufs=2 if db else 1)

        env_eaT_tile = eaT_tile

        env = dict(nc=nc, e1=e1, ea=ea, eatp=eatp, o1=o1, wqt=wqt_sb, wkt=wkt_sb,
                   wvb=wvb_sb, wotb=wotb_sb, idb=idb_sb, ones=ones_sb,
                   dumm=dumm_sb, magic=env_magic, eaT_tile=env_eaT_tile,
                   pP=pP, pA=pA, pBs=pBs, pC=pC,
                   psS=psS, psT=psT, psB=psB, psC=psC)

        def interleave(g1, g2):
            # g1 = prev rep's BC stream, g2 = next rep's A stream.
            # RATIO = BC yields consumed per A yield.
            ratio = int(_os.environ.get("IL_RATIO", "2"))
            alive = [g1, g2]
            while alive:
                for g in list(alive):
                    n = ratio if g is g1 else 1
                    for _ in range(n):
                        try:
                            next(g)
                        except StopIteration:
                            if g in alive:
                                alive.remove(g)
                            break

        def drain(g):
            for _ in g:
                pass

        prev_bc = None
        for _ in range(reps):
            st = make_state(env)
            ga = gen_phase_a(env, st)
            if prev_bc is None:
                drain(ga)
            else:
                interleave(prev_bc, ga)
            prev_bc = gen_phase_bc(env, st)
        drain(prev_bc)

    nc.compile()
    return nc


def make_state(env):
    pP, psS = env["pP"], env["psS"]
    st = {}
    st["s_sb"] = pP.tile([DQ, H, DK], F16, tag="s_sb", name="s_sb")
    st["sc_all"] = pP.tile([DQ, H, DK], F32, tag="sc_all", name="sc_all")
    st["e_all"] = pP.tile([DQ, H, DK], F16, tag="e_all", name="e_all")
    st["stats"] = pP.tile([DQ, H, 2], F32, tag="stats", name="stats")
    st["p2t"] = pP.tile([KHW, H, 2, DQ], F16, tag="p2t", name="p2t")
    st["junk"] = pP.tile([DQ, DK], F32, tag="junk", name="junk")
    for nm in ("mu", "m2", "mu2", "var", "sd", "rstd", "den", "rec",
               "nh", "t0", "t1"):
        st[nm] = pP.tile([DQ, H], F32, tag=nm, name=nm)
    st["ib"] = pP.tile([DQ, H], mybir.dt.int32, tag="ib", name="ib")
    st["s_ps"] = [psS.tile([DQ, 2, DK], F32, tag=f"s{g}", name=f"s_ps{g}")
                  for g in range(2)]
    # per-rep eaT tiles: host chunks may rotate between two buffers
    st["eaT"] = [env["eaT_tile"](nch) for nch in range(NCH)]
    return st


def gen_phase_a(env, st):
    """Loads, S accumulation, eaT transposes/loads. Yields after each block."""
    nc = env["nc"]
    pA, psT = env["pA"], env["psT"]
    e1, ea, idb_sb, eaT_c = env["e1"], env["ea"], env["idb"], st["eaT"]
    eatp = env["eatp"]
    s_ps = st["s_ps"]

    host_list = sorted(HOST_SET)
    if not RSQRT_DVE:
        # preload the sqrt act table while ACT is idle during phase A
        nc.scalar.activation(st["sd"][:, 0:1], env["dumm"][:], AF.Sqrt)
    for ii in range(NCH):
        e1b = pA.tile([PT, 4, C], F16, tag="e1b", name="e1b")
        nc.sync.dma_start(e1b[:], e1[ii])
        eab = pA.tile([PT, 4, KV], F16, tag="eab", name="eab")
        if _os.environ.get("EAB_ACT", "1") == "1":
            nc.scalar.dma_start(eab[:], ea[ii])
        else:
            nc.sync.dma_start(eab[:], ea[ii])
        host_t = ii in HOST_SET
        if host_t:
            # host-pre-transposed chunk: skip the PE transposes entirely;
            # spread these loads across the three DMA queues
            hi = host_list.index(ii)
            q = EATP_QS[hi % len(EATP_QS)]
            eng = {"s": nc.sync, "a": nc.scalar, "p": nc.gpsimd}[q]
            eng.dma_start(eaT_c[ii][:], eatp[hi])
        for a in range(4):
            i = 4 * ii + a
            for h in range(H):
                # two heads share a PSUM bank on disjoint column ranges.
                # start marks the whole bank pending-zero, so only the
                # first head's first matmul starts; the second head's
                # first write then overwrites those columns (correct).
                nc.tensor.matmul(
                    s_ps[h // 2][:, h % 2, :],
                    e1b[:, a, h * DQ:(h + 1) * DQ],
                    eab[:, a, h * DK:(h + 1) * DK],
                    start=(i == 0 and h % 2 == 0),
                    stop=(i == NT - 1 and h % 2 == 1),
                    skip_group_check=True,
                )
            if not host_t:
                # all 8 chunk-transposes of this 128-row tile batched into
                # one PSUM bank, then a single evac alternating DVE/ACT
                # (gpsimd cannot read PSUM on hardware)
                tp8 = psT.tile([CHW, DCH, PT], F16, tag="tp8", name="tp8")
                for j in range(DCH):
                    nc.tensor.transpose(
                        tp8[:, j, :], eab[:, a, j * CHW:(j + 1) * CHW],
                        idb_sb[:])
                if a % 2 == 0:
                    nc.vector.tensor_copy(
                        eaT_c[ii][:, :, a * PT:(a + 1) * PT], tp8[:])
                else:
                    nc.scalar.copy(
                        eaT_c[ii][:, :, a * PT:(a + 1) * PT], tp8[:])
        yield
    nc.scalar.copy(st["s_sb"][:, 0:2, :], s_ps[0][:])
    nc.vector.tensor_copy(st["s_sb"][:, 2:4, :], s_ps[1][:])


def gen_phase_bc(env, st):
    """Scores + instance-norm softmax (B), then context + output (C)."""
    nc = env["nc"]
    pBs, pC, psB, psC = env["pBs"], env["pC"], env["psB"], env["psC"]
    wqt_sb, wkt_sb, wvb_sb = env["wqt"], env["wkt"], env["wvb"]
    wotb_sb, idb_sb, ones_sb = env["wotb"], env["idb"], env["ones"]
    eaT_c, o1 = st["eaT"], env["o1"]
    s_sb, sc_all, e_all = st["s_sb"], st["sc_all"], st["e_all"]
    stats, p2t_sb, junk = st["stats"], st["p2t"], st["junk"]

    # --- phase B: scores + stats ------------------------------------------
    for h in range(H):
        # U.T = (S.T-chunks) @ (Wq_h.T/sqrt(KV))  [240k, 128e], fp16
        ut_sb = pBs.tile([KHW, KCH, DQ], F16, tag="ut_sb", name="ut_sb")
        for j in range(KCH):
            ut_ps = psB.tile([KHW, DQ], F32, tag="psb", name="ut_ps")
            nc.tensor.matmul(ut_ps[:], s_sb[:, h, j * KHW:(j + 1) * KHW],
                             wqt_sb[:, h, :], start=True, stop=True)
            nc.vector.tensor_copy(ut_sb[:, j, :], ut_ps[:])
        # scores = U @ Wk.T  [128e, 240ek], fp32 values via fp16 operands
        sc_ps = psB.tile([DQ, DK], F32, tag="psb", name="sc_ps")
        for j in range(KCH):
            nc.tensor.matmul(sc_ps[:], ut_sb[:, j, :], wkt_sb[:, j, :],
                             start=(j == 0), stop=(j == KCH - 1))
        # evacuate + per-row sums of x and x^2 for instance-norm; the x^2
        # path runs on DVE off the SBUF copy (two PSUM reads in one
        # tensor_tensor is illegal on hardware)
        nc.scalar.activation(sc_all[:, h, :], sc_ps[:], AF.Copy,
                             accum_out=stats[:, h, 0:1])
        nc.vector.tensor_tensor(junk[:], sc_all[:, h, :], sc_all[:, h, :],
                                ALU.mult)
        nc.vector.tensor_reduce(stats[:, h, 1:2], junk[:],
                                mybir.AxisListType.X, ALU.add)
        if h == 1:
            yield

    # cross-partition reduce of stats; every partition gets totals
    tot_ps = psB.tile([DQ, H, 2], F32, tag="psb", name="tot_ps")
    nc.tensor.matmul(tot_ps[:], ones_sb[:], stats[:], start=True, stop=True)
    nc.scalar.mul(st["mu"][:], tot_ps[:, :, 0:1], 1.0 / NORM_CNT)
    nc.scalar.mul(st["m2"][:], tot_ps[:, :, 1:2], 1.0 / NORM_CNT)
    nc.scalar.square(st["mu2"][:], st["mu"][:])
    nc.vector.tensor_sub(st["var"][:], st["m2"][:], st["mu2"][:])
    nc.vector.tensor_scalar_add(st["var"][:], st["var"][:], EPS)
    if RSQRT_DVE:
        # rstd = 1/sqrt(var) entirely on DVE: magic-constant seed + two
        # Newton iterations (~1e-6 rel).  Keeps Sqrt off ACT so the exp
        # table never gets evicted (saves two table loads per rep).
        var, ib = st["var"], st["ib"]
        nh, t0, t1, rstd = st["nh"], st["t0"], st["t1"], st["rstd"]
        magic = env["magic"]
        nc.vector.tensor_scalar(ib[:], var[:].bitcast(mybir.dt.int32), 1,
                                None, ALU.arith_shift_right)
        nc.vector.tensor_tensor(ib[:], magic[:], ib[:], ALU.subtract)
        nc.vector.tensor_scalar_mul(nh[:], var[:], -0.5)   # -v/2
        ys = [ib[:].bitcast(F32), st["sd"][:], rstd[:]]
        for y, ynext in zip(ys[:-1], ys[1:]):
            nc.vector.tensor_tensor(t0[:], y, y, ALU.mult)        # y^2
            nc.vector.tensor_tensor(t1[:], t0[:], nh[:], ALU.mult)
            nc.vector.tensor_scalar_add(t1[:], t1[:], 1.5)
            nc.vector.tensor_tensor(ynext, y, t1[:], ALU.mult)
    else:
        # sqrt_and_friends also holds Copy/Square, so with the per-rep dummy
        # sqrt preload only the later exp pays a table load
        nc.scalar.activation(st["sd"][:], st["var"][:], AF.Sqrt)
        nc.vector.reciprocal(st["rstd"][:], st["sd"][:])
    yield

    # softmax over ek of rstd*scores: the mean shift cancels in softmax,
    # and no max-shift is needed -- scores are z-scored by rstd so
    # |exponent| stays ~<=8, far from fp32 overflow. P2.T per head
    # immediately after that head's exp; the softmax denominator is
    # applied at the phase C ctx evac.
    for h in range(H):
        nc.scalar.activation(e_all[:, h, :], sc_all[:, h, :], AF.Exp,
                             scale=st["rstd"][:, h:h + 1],
                             accum_out=st["den"][:, h:h + 1])
        pt_sb = pBs.tile([KHW, KCH, DQ], F16, tag="pt_sb", name="pt_sb")
        for j in range(KCH):
            pt_ps = psB.tile([KHW, DQ], F16, tag="psb", name="pt_ps")
            nc.tensor.transpose(pt_ps[:], e_all[:, h, j * KHW:(j + 1) * KHW],
                                idb_sb[:])
            nc.vector.tensor_copy(pt_sb[:, j, :], pt_ps[:])
        for jd in range(2):
            p2t_ps = psB.tile([CHW, DQ], F32, tag="psb", name="p2t_ps")
            for jk in range(KCH):
                nc.tensor.matmul(p2t_ps[:],
                                 wvb_sb[:, jk, jd * CHW:(jd + 1) * CHW],
                                 pt_sb[:, jk, :],
                                 start=(jk == 0), stop=(jk == KCH - 1))
            nc.scalar.copy(p2t_sb[:, h, jd, :], p2t_ps[:])
        if h == 1:
            yield
    nc.vector.reciprocal(st["rec"][:], st["den"][:])
    yield

    # --- phase C: ctx_h = P2_h @ Kc_h.T, then O1 = sum_h ctx_h.T @ WoT_h --
    def emit_ctx(nch):
        ctx = pC.tile([DQ, H, 512], F16, tag="ctx", name="ctx", bufs=3)
        for h in range(H):
            cx_ps = psC.tile([DQ, 512], F32, tag="co", name="cx_ps")
            for jd in range(2):
                nc.tensor.matmul(cx_ps[:], p2t_sb[:, h, jd, :],
                                 eaT_c[nch][:, 2 * h + jd, :],
                                 start=(jd == 0), stop=(jd == 1))
            # evac applies the softmax denominator as a per-partition
            # scale, split across DVE and ACT (gpsimd cannot read PSUM)
            if h % 2 == 0:
                nc.vector.tensor_scalar_mul(ctx[:, h, :], cx_ps[:],
                                            st["rec"][:, h:h + 1])
            else:
                nc.scalar.activation(ctx[:, h, :], cx_ps[:], AF.Copy,
                                     scale=st["rec"][:, h:h + 1])
        return ctx

    def emit_oproj(nch, ctx):
        # batched output tile: all four 128-row tiles of this 512-token
        # block land in one SBUF tile, stored with a single DMA
        o_sb = pC.tile([PT, 4, C], F16, tag="o_sb", name="o_sb", bufs=2)
        for t in range(4):
            o_ps = psC.tile([PT, C], F32, tag="co", name="o_ps")
            for h in range(H):
                nc.tensor.matmul(o_ps[:], ctx[:, h, t * PT:(t + 1) * PT],
                                 wotb_sb[:, h, :],
                                 start=(h == 0), stop=(h == H - 1))
            if t % 2 == 0:
                nc.scalar.copy(o_sb[:, t, :], o_ps[:])
            else:
                nc.vector.tensor_copy(o_sb[:, t, :], o_ps[:])
            if not STORE_BATCH:
                eng = {"s": nc.sync, "a": nc.scalar, "p": nc.gpsimd}[STORE_Q]
                eng.dma_start(o1[nch][:, t, :], o_sb[:, t, :])
        if STORE_BATCH:
            eng = {"s": nc.sync, "a": nc.scalar, "p": nc.gpsimd}[STORE_Q]
            eng.dma_start(o1[nch], o_sb[:])

    prev = emit_ctx(0)
    yield
    for nch in range(1, NCH):
        cur = emit_ctx(nch)
        yield
        emit_oproj(nch - 1, prev)
        prev = cur
        yield
    emit_oproj(NCH - 1, prev)


_NC_CACHE = None


def get_nc():
    global _NC_CACHE
    if _NC_CACHE is None:
        _NC_CACHE = build_nc()
    return _NC_CACHE


def make_in_maps(emb1, emb_all, Wq, Wk, Wv, Wo):
    emb1 = np.asarray(emb1, dtype=np.float32)
    emb_all = np.asarray(emb_all, dtype=np.float32)
    Wq = np.asarray(Wq, dtype=np.float32)
    Wk = np.asarray(Wk, dtype=np.float32)
    Wv = np.asarray(Wv, dtype=np.float32)
    Wo = np.asarray(Wo, dtype=np.float32)

    # pre-tile to the DMA layout: row a*128+p of block ii -> [ii, p, a, ch]
    e1t = np.ascontiguousarray(
        emb1.reshape(B, NCH, 4, PT, C).transpose(0, 1, 3, 2, 4)
    ).astype(NP16)
    eat = np.ascontiguousarray(
        emb_all.reshape(B, NCH, 4, PT, KV).transpose(0, 1, 3, 2, 4)
    ).astype(NP16)

    scale = 1.0 / np.sqrt(np.float32(KV))
    wqt_np = np.ascontiguousarray(
        np.transpose(Wq, (2, 0, 1)) * scale).astype(NP16)  # [c,h,e]
    wkt_np = np.ascontiguousarray(Wk.T).astype(NP16)       # [k,ek]
    wvb_np = np.ascontiguousarray(Wv).astype(NP16)         # [k,d]
    wotb_np = np.ascontiguousarray(
        Wo.reshape(C, DQ, H).transpose(1, 2, 0)).astype(NP16)
    idb_np = np.eye(PT, dtype=NP16)

    # host-pre-transposed eaT chunks: [nch][kv%120][kv//120][token]
    host_list = sorted(HOST_SET)
    if host_list:
        eatp_np = np.empty((B, len(host_list), CHW, DCH, 512),
                           dtype=NP16)
        ea_k = emb_all.astype(NP16)
        for x, nch in enumerate(host_list):
            blk = ea_k[:, nch * 512:(nch + 1) * 512, :]       # [B, 512, KV]
            eatp_np[:, x] = blk.reshape(B, 512, DCH, CHW).transpose(0, 3, 2, 1)
    else:
        eatp_np = np.zeros((B, 1, CHW, DCH, 512), dtype=NP16)

    shared = {"wqt": wqt_np, "wkt": wkt_np, "wvb": wvb_np, "wotb": wotb_np,
              "idb": idb_np}
    return [
        {"e1": e1t[b], "ea": eat[b], "eatp": eatp_np[b], **shared}
        for b in range(B)
    ]


def run(inputs, trace=False, **spmd_kwargs):
    nc = get_nc()
    in_maps = make_in_maps(**inputs)
    res = run_bass_kernel_spmd(nc, in_maps, list(range(B)), trace=trace,
                               **spmd_kwargs)
    # o1 comes back block-tiled [NCH, PT, 4, C]; untile to [N, C]
    out = np.stack(
        [np.asarray(res.results[b]["o1"]).transpose(0, 2, 1, 3).reshape(N, C)
         for b in range(B)], axis=0)
    return out.astype(np.float32), res


def kernel(**inputs) -> np.ndarray:
    out, _ = run(inputs, trace=False)
    return out


# revision 24
# speedup vs baseline: 1.0256x; 1.0256x over previous
"""Trainium2 Bass kernel for nn_Attention_org_single_85074712199391.

Channel-attention module. Reference math (per batch b, head h):
    Qc = emb1[b].reshape(N, 4, dq)[:, h]          # [N, 128]
    Kc = emb_all[b].reshape(N, 4, dk)[:, h]       # [N, 240]
    Q = Qc @ Wq[h].T ; K = Kc @ Wk.T ; V = Kc @ Wv.T
    scores = Q.T @ K / sqrt(KV)                   # [128, 240]
    probs = softmax(instnorm(scores), axis=-1)
    context = probs @ V.T                         # [128, N]
    O1 = permute/concat(context) @ Wo.T           # [N, 512]

Algebraic rewrite used here (exact):
    S_h      = Qc.T @ Kc                          # big contraction over N
    scores_h = (Wq[h]/sqrt(KV)) @ S_h @ Wk.T
    probs_h  = softmax over dk of rstd*scores_h   # mean cancels in softmax
    P2_h     = probs_h @ Wv                       # [128, 240], unnormalized
    ctx_h    = (P2_h @ Kc.T) / den_h              # [128, N]
    O1       = sum_h ctx_h.T @ Wo[:, h::4].T      # accumulate over heads

Per core (core b owns batch b; weights replicated; no collectives):
    A: stream e1/ea (host-converted fp16, pre-tiled); accumulate S in two
       2-head-packed PSUM banks.  The transposed-ea tiles (eaT) that phase C
       needs come from a MIX of host-pre-transposed DMA loads (HOST_EAT
       chunks, spread across the SP/ACT/gpsimd DMA queues) and on-chip PE
       transposes (remaining chunks; PSUM evac alternates DVE/ACT).
    B: fp16 scores-path matmuls (S/Wq/Wk/U quantized to fp16; the score
       values themselves stay fp32 through PSUM/SBUF so the softmax input
       is accurate); instance-norm stats via ones-matmul with the x^2 row
       sums on DVE; softmax denominator deferred to the phase C ctx evac.
    C: context matmuls off eaT, output projection accumulating over heads,
       fp16 stores batched per 512-token block on the gpsimd (SWDGE) queue.

Rep pipelining: rep r+1's phase A instruction stream is emitted
interleaved with rep r's phases B+C, so each engine's in-order queue
always holds cross-rep work while serial chains (instance-norm stats,
softmax) block the current rep. eaT chunk tiles give write-after-read
dependencies at chunk granularity (phase A block ii <-> phase C chunk ii).
"""

import sys

import numpy as np

try:
    import concourse.bass as bass
except ImportError:  # harness environments without the repo on sys.path
    sys.path.insert(0, "/opt/trn_rl_repo")
    import concourse.bass as bass

import concourse.bacc as bacc

import concourse.mybir as mybir
import concourse.tile as tile
from concourse.bass_utils import run_bass_kernel_spmd

import os as _os0

F32 = mybir.dt.float32
if _os0.environ.get("DT16", "f16") == "bf16":
    import ml_dtypes
    F16 = mybir.dt.bfloat16
    NP16 = ml_dtypes.bfloat16
else:
    F16 = mybir.dt.float16
    NP16 = np.float16
AF = mybir.ActivationFunctionType
ALU = mybir.AluOpType

B, N, C, KV, H = 8, 4096, 512, 960, 4
DQ, DK = C // 4, KV // 4          # 128, 240
PT = 128                          # partition tile
NT = N // PT                      # 32 row tiles
NCH = N // 512                    # 8 column chunks / token blocks
DCH = 8                           # KV split into 8 chunks of 120 partitions
CHW = KV // DCH                   # 120
KCH = 2                           # dk split for 240-deep contractions
KHW = DK // KCH                   # 120
EPS = 1e-5
NORM_CNT = float(DQ * DK)         # instance-norm element count

import os as _os
# token blocks whose eaT chunk is host-pre-transposed and DMA-loaded
# (trades spare DMA bandwidth for PE transpose + PSUM-evac time)
HOST_EAT = int(_os.environ.get("HOST_EAT", "4"))
HOST_SET = {round((2 * k + 1) * NCH / (2 * HOST_EAT) - 0.5) for k in range(HOST_EAT)} if HOST_EAT else set()
# DMA queue per host-transposed chunk, cycled: s=SP, a=ACT, p=gpsimd/Pool
EATP_QS = _os.environ.get("EATP_QS", "spsp")
# queue for the batched per-block output stores: s/a/p
STORE_Q = _os.environ.get("STORE_Q", "p")
# compute rstd = 1/sqrt(var) on DVE (magic-seed Newton) instead of ACT Sqrt;
# keeps the ACT exp table resident forever (no per-rep table churn)
RSQRT_DVE = _os.environ.get("RSQRT_DVE", "0") == "1"
# double-buffer the host-loaded eaT chunks so rep r+1's eatp DMA never
# waits on rep r's phase-C reads (costs 8KB/partition per chunk)
EAT_DB = _os.environ.get("EAT_DB", "0") == "1"
# 1 = one DMA per 512-token block; 0 = one DMA per 128-row tile (4x more)
STORE_BATCH = _os.environ.get("STORE_BATCH", "1") == "1"
# eaT PSUM evacuation: 1 = alternate DVE/ACT, 0 = all DVE (ACT queue also
# issues the ea loads, so ACT evacs can delay them)
EVAC_ALT = _os.environ.get("EVAC_ALT", "1") == "1"


def build_nc(reps=1):
    nc = bacc.Bacc("TRN2", target_bir_lowering=False, debug=False)

    # pre-tiled fp16 inputs: [8 blocks][128 partitions][4 subtiles][ch]
    e1 = nc.dram_tensor("e1", [NCH, PT, 4, C], F16, kind="ExternalInput").ap()
    ea = nc.dram_tensor("ea", [NCH, PT, 4, KV], F16, kind="ExternalInput").ap()
    wqt = nc.dram_tensor("wqt", [DQ, H, DQ], F16, kind="ExternalInput").ap()
    wkt = nc.dram_tensor("wkt", [DK, DK], F16, kind="ExternalInput").ap()
    wvb = nc.dram_tensor("wvb", [DK, DK], F16, kind="ExternalInput").ap()
    wotb = nc.dram_tensor("wotb", [DQ, H, C], F16, kind="ExternalInput").ap()
    idb = nc.dram_tensor("idb", [PT, PT], F16, kind="ExternalInput").ap()
    eatp = nc.dram_tensor("eatp", [max(len(HOST_SET), 1), CHW, DCH, 512],
                          F16, kind="ExternalInput").ap()
    # output stored block-tiled: token n = nch*512 + t*128 + p -> [nch, p, t, :]
    o1 = nc.dram_tensor("o1", [NCH, PT, 4, C], F16, kind="ExternalOutput").ap()

    from contextlib import ExitStack

    with tile.TileContext(nc) as tc, ExitStack() as stk:
        ent = stk.enter_context
        pW = ent(tc.tile_pool(name="weights", bufs=1))
        pEAT = ent(tc.tile_pool(name="eaTbuf", bufs=1))
        pP = ent(tc.tile_pool(name="persist", bufs=2))
        pA = ent(tc.tile_pool(name="pA", bufs=int(_os.environ.get('PA_BUFS', '7'))))
        pBs = ent(tc.tile_pool(name="pBs", bufs=int(_os.environ.get('PBS_BUFS', '2'))))
        pC = ent(tc.tile_pool(name="pC", bufs=int(_os.environ.get('PC_BUFS', '3'))))
        psS = ent(tc.tile_pool(name="psS", bufs=1, space="PSUM"))
        psT = ent(tc.tile_pool(name="psT", bufs=int(_os.environ.get('PST_BUFS','2')), space="PSUM"))
        psB = ent(tc.tile_pool(name="psB", bufs=int(_os.environ.get('PSB_BUFS','2')), space="PSUM"))
        psC = ent(tc.tile_pool(name="psC", bufs=int(_os.environ.get('PSC_BUFS','2')), space="PSUM"))

        # --- persistent weights / constants (loaded once) -----------------
        wqt_sb = pW.tile([DQ, H, DQ], F16, tag="wqt_sb")
        nc.sync.dma_start(wqt_sb[:], wqt[:])
        wkt_sb = pW.tile([KHW, KCH, DK], F16, tag="wkt_sb")
        wvb_sb = pW.tile([KHW, KCH, DK], F16, tag="wvb_sb")
        for j in range(KCH):
            nc.sync.dma_start(wkt_sb[:, j, :], wkt[j * KHW:(j + 1) * KHW, :])
            nc.sync.dma_start(wvb_sb[:, j, :], wvb[j * KHW:(j + 1) * KHW, :])
        wotb_sb = pW.tile([DQ, H, C], F16, tag="wotb_sb")
        nc.sync.dma_start(wotb_sb[:], wotb[:])
        idb_sb = pW.tile([PT, PT], F16, tag="idb_sb")
        nc.sync.dma_start(idb_sb[:], idb[:])
        ones_sb = pW.tile([PT, PT], F32, tag="ones_sb")
        nc.vector.memset(ones_sb[:], 1.0)
        dumm_sb = pW.tile([PT, 1], F32, tag="dumm_sb")
        nc.vector.memset(dumm_sb[:], 1.0)
        expd_sb = pW.tile([PT, 1], F32, tag="expd_sb")
        # one-time exp-table preload; every ACT func used afterwards
        # (Copy/Square/Exp) lives in the exp_and_friends table
        nc.scalar.activation(expd_sb[:], dumm_sb[:], AF.Exp)
        magic_sb = pW.tile([DQ, H], mybir.dt.int32, tag="magic_sb")
        # 0x5F3759DF: rsqrt magic seed (see make_state/gen_phase_bc)
        nc.vector.memset(magic_sb[:], 0x5F3759DF)
        env_magic = magic_sb

        # eaT is split per 512-token chunk: phase A block ii fills chunk ii
        # and phase C chunk ii is its only reader, so rep r+1's writes only
        # wait for rep r's same-chunk reads.  Host-loaded chunks may be
        # double-buffered (EAT_DB) to decouple that dependency entirely.
        def eaT_tile(nch):
            db = EAT_DB and nch in HOST_SET
            return pEAT.tile([CHW, DCH, 512], F16, tag=f"eaT{nch}",
                             name=f"eaT{nch}", bufs=2 if db else 1)

        env_eaT_tile = eaT_tile

        env = dict(nc=nc, e1=e1, ea=ea, eatp=eatp, o1=o1, wqt=wqt_sb, wkt=wkt_sb,
                   wvb=wvb_sb, wotb=wotb_sb, idb=idb_sb, ones=ones_sb,
                   dumm=dumm_sb, magic=env_magic, eaT_tile=env_eaT_tile,
                   pP=pP, pA=pA, pBs=pBs, pC=pC,
                   psS=psS, psT=psT, psB=psB, psC=psC)

        def interleave(g1, g2):
            # g1 = prev rep's BC stream, g2 = next rep's A stream.
            # RATIO = BC yields consumed per A yield.
            ratio = int(_os.environ.get("IL_RATIO", "2"))
            alive = [g1, g2]
            while alive:
                for g in list(alive):
                    n = ratio if g is g1 else 1
                    for _ in range(n):
                        try:
                            next(g)
                        except StopIteration:
                            if g in alive:
                                alive.remove(g)
                            break

        def drain(g):
            for _ in g:
                pass

        prev_bc = None
        for _ in range(reps):
            st = make_state(env)
            ga = gen_phase_a(env, st)
            if prev_bc is None:
                drain(ga)
            else:
                interleave(prev_bc, ga)
            prev_bc = gen_phase_bc(env, st)
        drain(prev_bc)

    nc.compile()
    return nc


def make_state(env):
    pP, psS = env["pP"], env["psS"]
    st = {}
    st["s_sb"] = pP.tile([DQ, H, DK], F16, tag="s_sb", name="s_sb")
    st["sc_all"] = pP.tile([DQ, H, DK], F32, tag="sc_all", name="sc_all")
    st["e_all"] = pP.tile([DQ, H, DK], F16, tag="e_all", name="e_all")
    st["stats"] = pP.tile([DQ, H, 2], F32, tag="stats", name="stats")
    st["p2t"] = pP.tile([KHW, H, 2, DQ], F16, tag="p2t", name="p2t")
    st["junk"] = pP.tile([DQ, DK], F32, tag="junk", name="junk")
    for nm in ("mu", "m2", "mu2", "var", "sd", "rstd", "den", "rec",
               "nh", "t0", "t1"):
        st[nm] = pP.tile([DQ, H], F32, tag=nm, name=nm)
    st["ib"] = pP.tile([DQ, H], mybir.dt.int32, tag="ib", name="ib")
    st["s_ps"] = [psS.tile([DQ, 2, DK], F32, tag=f"s{g}", name=f"s_ps{g}")
                  for g in range(2)]
    # per-rep eaT tiles: host chunks may rotate between two buffers
    st["eaT"] = [env["eaT_tile"](nch) for nch in range(NCH)]
    return st


def gen_phase_a(env, st):
    """Loads, S accumulation, eaT transposes/loads. Yields after each block."""
    nc = env["nc"]
    pA, psT = env["pA"], env["psT"]
    e1, ea, idb_sb, eaT_c = env["e1"], env["ea"], env["idb"], st["eaT"]
    eatp = env["eatp"]
    s_ps = st["s_ps"]

    host_list = sorted(HOST_SET)
    if not RSQRT_DVE:
        # preload the sqrt act table while ACT is idle during phase A
        nc.scalar.activation(st["sd"][:, 0:1], env["dumm"][:], AF.Sqrt)
    for ii in range(NCH):
        e1b = pA.tile([PT, 4, C], F16, tag="e1b", name="e1b")
        nc.sync.dma_start(e1b[:], e1[ii])
        eab = pA.tile([PT, 4, KV], F16, tag="eab", name="eab")
        if _os.environ.get("EAB_ACT", "1") == "1":
            nc.scalar.dma_start(eab[:], ea[ii])
        else:
            nc.sync.dma_start(eab[:], ea[ii])
        host_t = ii in HOST_SET
        if host_t:
            # host-pre-transposed chunk: skip the PE transposes entirely;
            # spread these loads across the three DMA queues
            hi = host_list.index(ii)
            q = EATP_QS[hi % len(EATP_QS)]
            eng = {"s": nc.sync, "a": nc.scalar, "p": nc.gpsimd}[q]
            eng.dma_start(eaT_c[ii][:], eatp[hi])
        for a in range(4):
            i = 4 * ii + a
            for h in range(H):
                # two heads share a PSUM bank on disjoint column ranges.
                # start marks the whole bank pending-zero, so only the
                # first head's first matmul starts; the second head's
                # first write then overwrites those columns (correct).
                nc.tensor.matmul(
                    s_ps[h // 2][:, h % 2, :],
                    e1b[:, a, h * DQ:(h + 1) * DQ],
                    eab[:, a, h * DK:(h + 1) * DK],
                    start=(i == 0 and h % 2 == 0),
                    stop=(i == NT - 1 and h % 2 == 1),
                    skip_group_check=True,
                )
            if not host_t:
                # all 8 chunk-transposes of this 128-row tile batched into
                # one PSUM bank, then a single evac alternating DVE/ACT
                # (gpsimd cannot read PSUM on hardware)
                tp8 = psT.tile([CHW, DCH, PT], F16, tag="tp8", name="tp8")
                for j in range(DCH):
                    nc.tensor.transpose(
                        tp8[:, j, :], eab[:, a, j * CHW:(j + 1) * CHW],
                        idb_sb[:])
                if EVAC_ALT and a % 2 == 1:
                    nc.scalar.copy(
                        eaT_c[ii][:, :, a * PT:(a + 1) * PT], tp8[:])
                else:
                    nc.vector.tensor_copy(
                        eaT_c[ii][:, :, a * PT:(a + 1) * PT], tp8[:])
        yield
    nc.scalar.copy(st["s_sb"][:, 0:2, :], s_ps[0][:])
    nc.vector.tensor_copy(st["s_sb"][:, 2:4, :], s_ps[1][:])


def gen_phase_bc(env, st):
    """Scores + instance-norm softmax (B), then context + output (C)."""
    nc = env["nc"]
    pBs, pC, psB, psC = env["pBs"], env["pC"], env["psB"], env["psC"]
    wqt_sb, wkt_sb, wvb_sb = env["wqt"], env["wkt"], env["wvb"]
    wotb_sb, idb_sb, ones_sb = env["wotb"], env["idb"], env["ones"]
    eaT_c, o1 = st["eaT"], env["o1"]
    s_sb, sc_all, e_all = st["s_sb"], st["sc_all"], st["e_all"]
    stats, p2t_sb, junk = st["stats"], st["p2t"], st["junk"]

    # --- phase B: scores + stats ------------------------------------------
    for h in range(H):
        # U.T = (S.T-chunks) @ (Wq_h.T/sqrt(KV))  [240k, 128e], fp16
        ut_sb = pBs.tile([KHW, KCH, DQ], F16, tag="ut_sb", name="ut_sb")
        for j in range(KCH):
            ut_ps = psB.tile([KHW, DQ], F32, tag="psb", name="ut_ps")
            nc.tensor.matmul(ut_ps[:], s_sb[:, h, j * KHW:(j + 1) * KHW],
                             wqt_sb[:, h, :], start=True, stop=True)
            nc.vector.tensor_copy(ut_sb[:, j, :], ut_ps[:])
        # scores = U @ Wk.T  [128e, 240ek], fp32 values via fp16 operands
        sc_ps = psB.tile([DQ, DK], F32, tag="psb", name="sc_ps")
        for j in range(KCH):
            nc.tensor.matmul(sc_ps[:], ut_sb[:, j, :], wkt_sb[:, j, :],
                             start=(j == 0), stop=(j == KCH - 1))
        # evacuate + per-row sums of x and x^2 for instance-norm; the x^2
        # path runs on DVE off the SBUF copy (two PSUM reads in one
        # tensor_tensor is illegal on hardware)
        nc.scalar.activation(sc_all[:, h, :], sc_ps[:], AF.Copy,
                             accum_out=stats[:, h, 0:1])
        nc.vector.tensor_tensor(junk[:], sc_all[:, h, :], sc_all[:, h, :],
                                ALU.mult)
        nc.vector.tensor_reduce(stats[:, h, 1:2], junk[:],
                                mybir.AxisListType.X, ALU.add)
        if h == 1:
            yield

    # cross-partition reduce of stats; every partition gets totals
    tot_ps = psB.tile([DQ, H, 2], F32, tag="psb", name="tot_ps")
    nc.tensor.matmul(tot_ps[:], ones_sb[:], stats[:], start=True, stop=True)
    nc.scalar.mul(st["mu"][:], tot_ps[:, :, 0:1], 1.0 / NORM_CNT)
    nc.scalar.mul(st["m2"][:], tot_ps[:, :, 1:2], 1.0 / NORM_CNT)
    nc.scalar.square(st["mu2"][:], st["mu"][:])
    nc.vector.tensor_sub(st["var"][:], st["m2"][:], st["mu2"][:])
    nc.vector.tensor_scalar_add(st["var"][:], st["var"][:], EPS)
    if RSQRT_DVE:
        # rstd = 1/sqrt(var) entirely on DVE: magic-constant seed + two
        # Newton iterations (~1e-6 rel).  Keeps Sqrt off ACT so the exp
        # table never gets evicted (saves two table loads per rep).
        var, ib = st["var"], st["ib"]
        nh, t0, t1, rstd = st["nh"], st["t0"], st["t1"], st["rstd"]
        magic = env["magic"]
        nc.vector.tensor_scalar(ib[:], var[:].bitcast(mybir.dt.int32), 1,
                                None, ALU.arith_shift_right)
        nc.vector.tensor_tensor(ib[:], magic[:], ib[:], ALU.subtract)
        nc.vector.tensor_scalar_mul(nh[:], var[:], -0.5)   # -v/2
        ys = [ib[:].bitcast(F32), st["sd"][:], rstd[:]]
        for y, ynext in zip(ys[:-1], ys[1:]):
            nc.vector.tensor_tensor(t0[:], y, y, ALU.mult)        # y^2
            nc.vector.tensor_tensor(t1[:], t0[:], nh[:], ALU.mult)
            nc.vector.tensor_scalar_add(t1[:], t1[:], 1.5)
            nc.vector.tensor_tensor(ynext, y, t1[:], ALU.mult)
    else:
        # sqrt_and_friends also holds Copy/Square, so with the per-rep dummy
        # sqrt preload only the later exp pays a table load
        nc.scalar.activation(st["sd"][:], st["var"][:], AF.Sqrt)
        nc.vector.reciprocal(st["rstd"][:], st["sd"][:])
    yield

    # softmax over ek of rstd*scores: the mean shift cancels in softmax,
    # and no max-shift is needed -- scores are z-scored by rstd so
    # |exponent| stays ~<=8, far from fp32 overflow. P2.T per head
    # immediately after that head's exp; the softmax denominator is
    # applied at the phase C ctx evac.
    for h in range(H):
        nc.scalar.activation(e_all[:, h, :], sc_all[:, h, :], AF.Exp,
                             scale=st["rstd"][:, h:h + 1],
                             accum_out=st["den"][:, h:h + 1])
        pt_sb = pBs.tile([KHW, KCH, DQ], F16, tag="pt_sb", name="pt_sb")
        for j in range(KCH):
            pt_ps = psB.tile([KHW, DQ], F16, tag="psb", name="pt_ps")
            nc.tensor.transpose(pt_ps[:], e_all[:, h, j * KHW:(j + 1) * KHW],
                                idb_sb[:])
            nc.vector.tensor_copy(pt_sb[:, j, :], pt_ps[:])
        for jd in range(2):
            p2t_ps = psB.tile([CHW, DQ], F32, tag="psb", name="p2t_ps")
            for jk in range(KCH):
                nc.tensor.matmul(p2t_ps[:],
                                 wvb_sb[:, jk, jd * CHW:(jd + 1) * CHW],
                                 pt_sb[:, jk, :],
                                 start=(jk == 0), stop=(jk == KCH - 1))
            nc.scalar.copy(p2t_sb[:, h, jd, :], p2t_ps[:])
        if h == 1:
            yield
    nc.vector.reciprocal(st["rec"][:], st["den"][:])
    yield

    # --- phase C: ctx_h = P2_h @ Kc_h.T, then O1 = sum_h ctx_h.T @ WoT_h --
    def emit_ctx(nch):
        ctx = pC.tile([DQ, H, 512], F16, tag="ctx", name="ctx", bufs=3)
        for h in range(H):
            cx_ps = psC.tile([DQ, 512], F32, tag="co", name="cx_ps")
            for jd in range(2):
                nc.tensor.matmul(cx_ps[:], p2t_sb[:, h, jd, :],
                                 eaT_c[nch][:, 2 * h + jd, :],
                                 start=(jd == 0), stop=(jd == 1))
            # evac applies the softmax denominator as a per-partition
            # scale, split across DVE and ACT (gpsimd cannot read PSUM)
            if h % 2 == 0:
                nc.vector.tensor_scalar_mul(ctx[:, h, :], cx_ps[:],
                                            st["rec"][:, h:h + 1])
            else:
                nc.scalar.activation(ctx[:, h, :], cx_ps[:], AF.Copy,
                                     scale=st["rec"][:, h:h + 1])
        return ctx

    def emit_oproj(nch, ctx):
        # batched output tile: all four 128-row tiles of this 512-token
        # block land in one SBUF tile, stored with a single DMA
        o_sb = pC.tile([PT, 4, C], F16, tag="o_sb", name="o_sb", bufs=2)
        for t in range(4):
            o_ps = psC.tile([PT, C], F32, tag="co", name="o_ps")
            for h in range(H):
                nc.tensor.matmul(o_ps[:], ctx[:, h, t * PT:(t + 1) * PT],
                                 wotb_sb[:, h, :],
                                 start=(h == 0), stop=(h == H - 1))
            if t % 2 == 0:
                nc.scalar.copy(o_sb[:, t, :], o_ps[:])
            else:
                nc.vector.tensor_copy(o_sb[:, t, :], o_ps[:])
            if not STORE_BATCH:
                eng = {"s": nc.sync, "a": nc.scalar, "p": nc.gpsimd}[STORE_Q]
                eng.dma_start(o1[nch][:, t, :], o_sb[:, t, :])
        if STORE_BATCH:
            eng = {"s": nc.sync, "a": nc.scalar, "p": nc.gpsimd}[STORE_Q]
            eng.dma_start(o1[nch], o_sb[:])

    prev = emit_ctx(0)
    yield
    for nch in range(1, NCH):
        cur = emit_ctx(nch)
        yield
        emit_oproj(nch - 1, prev)
        prev = cur
        yield
    emit_oproj(NCH - 1, prev)


_NC_CACHE = None


def get_nc():
    global _NC_CACHE
    if _NC_CACHE is None:
        _NC_CACHE = build_nc()
    return _NC_CACHE


def make_in_maps(emb1, emb_all, Wq, Wk, Wv, Wo):
    emb1 = np.asarray(emb1, dtype=np.float32)
    emb_all = np.asarray(emb_all, dtype=np.float32)
    Wq = np.asarray(Wq, dtype=np.float32)
    Wk = np.asarray(Wk, dtype=np.float32)
    Wv = np.asarray(Wv, dtype=np.float32)
    Wo = np.asarray(Wo, dtype=np.float32)

    # pre-tile to the DMA layout: row a*128+p of block ii -> [ii, p, a, ch]
    e1t = np.ascontiguousarray(
        emb1.reshape(B, NCH, 4, PT, C).transpose(0, 1, 3, 2, 4)
    ).astype(NP16)
    eat = np.ascontiguousarray(
        emb_all.reshape(B, NCH, 4, PT, KV).transpose(0, 1, 3, 2, 4)
    ).astype(NP16)

    scale = 1.0 / np.sqrt(np.float32(KV))
    wqt_np = np.ascontiguousarray(
        np.transpose(Wq, (2, 0, 1)) * scale).astype(NP16)  # [c,h,e]
    wkt_np = np.ascontiguousarray(Wk.T).astype(NP16)       # [k,ek]
    wvb_np = np.ascontiguousarray(Wv).astype(NP16)         # [k,d]
    wotb_np = np.ascontiguousarray(
        Wo.reshape(C, DQ, H).transpose(1, 2, 0)).astype(NP16)
    idb_np = np.eye(PT, dtype=NP16)

    # host-pre-transposed eaT chunks: [nch][kv%120][kv//120][token]
    host_list = sorted(HOST_SET)
    if host_list:
        eatp_np = np.empty((B, len(host_list), CHW, DCH, 512),
                           dtype=NP16)
        ea_k = emb_all.astype(NP16)
        for x, nch in enumerate(host_list):
            blk = ea_k[:, nch * 512:(nch + 1) * 512, :]       # [B, 512, KV]
            eatp_np[:, x] = blk.reshape(B, 512, DCH, CHW).transpose(0, 3, 2, 1)
    else:
        eatp_np = np.zeros((B, 1, CHW, DCH, 512), dtype=NP16)

    shared = {"wqt": wqt_np, "wkt": wkt_np, "wvb": wvb_np, "wotb": wotb_np,
              "idb": idb_np}
    return [
        {"e1": e1t[b], "ea": eat[b], "eatp": eatp_np[b], **shared}
        for b in range(B)
    ]


def run(inputs, trace=False, **spmd_kwargs):
    nc = get_nc()
    in_maps = make_in_maps(**inputs)
    res = run_bass_kernel_spmd(nc, in_maps, list(range(B)), trace=trace,
                               **spmd_kwargs)
    # o1 comes back block-tiled [NCH, PT, 4, C]; untile to [N, C]
    out = np.stack(
        [np.asarray(res.results[b]["o1"]).transpose(0, 2, 1, 3).reshape(N, C)
         for b in range(B)], axis=0)
    return out.astype(np.float32), res


def kernel(**inputs) -> np.ndarray:
    out, _ = run(inputs, trace=False)
    return out
